# revision 2
# baseline (speedup 1.0000x reference)
"""Trainium2 Bass kernel v2 for nn_CircuitChannel — Y-basis real-gate pipeline.

Math: RX(t) = S R(t) S^dag per qubit with S = diag(1, i), R = [[c, s], [-s, c]]
real. The full circuit CZ*RX4*CZ*RX3*CZ*RX2*CZ*RX1 equals
S * [CZ R4 CZ R3 CZ R2 CZ R1] * S^dag (S telescopes; S CZ S^dag = CZ).

Host: chi = S^dag psi (diag (-i)^popcount).
Device: phi = R4' ... R1' chi where all gates are REAL 128x128 7-qubit-group
matrices (half the PE streaming of the complex baseline); inter-layer CZ signs
are folded into next-layer gate matrices (pairs co-located in a stage window)
or applied as per-partition-vector scaled PSUM evacuations (cross-window
pairs). Final CZ + S are applied on the host after the device measurement
(both are unit-modulus diagonals, so the qubit-0 probability measurement,
masking and normalization commute with them).

Stage structure (TM6/TM0 transposing matmuls + PM) and the measurement/output
stages follow the v1 baseline.
"""
import sys
sys.path.insert(0, "/opt/trn_rl_repo")
import numpy as np

N = 20
DIM = 1 << N
BATCH = 8
NLAYERS = 4

STAGES = [
    ("TM6", 0), ("TM0", 0), ("PM", 0),
    ("TM6", 1), ("TM0", 1), ("PM", 1),
    ("TM0", 2), ("TM6", 2), ("PM", 2),
    ("TM0", 3), ("TM6", 3), ("PM", 3),
]

STATE_BF16 = True  # state + weights dtype on device

CZ_PAIRS = [(q, q + 1) for q in range(N - 1)]


# ------------------------- host-side plan -------------------------

def _ry(theta):
    c, s = np.cos(theta / 2), np.sin(theta / 2)
    return np.array([[c, s], [-s, c]], dtype=np.float64)


def _apply_sigma(layout, t):
    l = list(layout)
    if t == 6:
        return l[13:20] + l[7:13] + l[0:7]
    return l[7:14] + l[0:7] + l[14:20]


def build_plan_v2(thetas):
    layout = list(range(N))
    done = set()
    cur_layer = -1
    metas = []
    for s, (stype, layer) in enumerate(STAGES):
        if layer != cur_layer:
            done = set()
            cur_layer = layer
        win = list(layout[0:7])
        fresh = [q for q in win if q not in done]
        done.update(win)
        in_layout = list(layout)
        if stype == "TM6":
            layout = _apply_sigma(layout, 6)
        elif stype == "TM0":
            layout = _apply_sigma(layout, 0)
        metas.append(dict(s=s, stype=stype, layer=layer, win=win, fresh=fresh,
                          in_layout=in_layout, out_layout=list(layout)))
    assert layout == list(range(N))

    gate_stage = {}
    for m in metas:
        for q in m["fresh"]:
            gate_stage[(m["layer"], q)] = m["s"]

    fold_into_stage = {s: [] for s in range(len(STAGES))}
    evac_pairs = {s: [] for s in range(len(STAGES))}
    for l in range(NLAYERS - 1):
        for (a, b) in CZ_PAIRS:
            sa, sb = gate_stage[(l + 1, a)], gate_stage[(l + 1, b)]
            if sa == sb:
                fold_into_stage[sa].append((a, b))
            else:
                lo = max(gate_stage[(l, a)], gate_stage[(l, b)])
                hi = min(sa, sb)
                best = None
                for s in range(lo, hi):
                    ol = metas[s]["out_layout"]
                    pa, pb = ol.index(a), ol.index(b)
                    fbits = tuple(sorted(p - 7 for p in (pa, pb) if p >= 7))
                    pbits = tuple(sorted(p for p in (pa, pb) if p < 7))
                    cost = sum(1 if (1 << (12 - f)) >= 256 else 2
                               for f in fbits)
                    cand = (cost, -min([1 << (12 - f) for f in fbits],
                                       default=8192), s)
                    if best is None or cand < best:
                        best = cand + ((pbits, fbits),)
                assert best is not None, (l, a, b, lo, hi)
                evac_pairs[best[2]].append((a, b) + (best[3],))

    for m in metas:
        s, layer, win, fresh = m["s"], m["layer"], m["win"], m["fresh"]
        U = np.array([[1.0]])
        for q in win:
            g = _ry(thetas[layer, q]) if q in fresh else np.eye(2)
            U = np.kron(U, g)
        for (a, b) in fold_into_stage[s]:
            ia, ib = win.index(a), win.index(b)
            idx = np.arange(128)
            d = 1.0 - 2.0 * (((idx >> (6 - ia)) & 1) & ((idx >> (6 - ib)) & 1))
            U = U * d[None, :]  # U @ diag(d)
        m["U"] = U
        m["evac_pairs"] = evac_pairs[s]
    return metas


def stage_decorations(meta):
    """Evac sign decoration for one stage.

    Returns (split_bits, region_vecs):
      split_bits: sorted list of output-free-axis bit positions (0 = MSB of
        13-bit free index) involved in any pair at this stage.
      region_vecs: dict mapping region key (tuple of bit values, same order as
        split_bits) -> np.float32 [128] per-partition sign vector (or None if
        all ones).
    """
    pairs = meta["evac_pairs"]
    split_bits = sorted({f for (_a, _b, (pb, fb)) in pairs for f in fb})
    p = np.arange(128)
    region_vecs = {}
    nb = len(split_bits)
    for key in range(1 << nb):
        bits = {split_bits[i]: (key >> (nb - 1 - i)) & 1 for i in range(nb)}
        vec = np.ones(128, dtype=np.float64)
        for (_a, _b, (pbits, fbits)) in pairs:
            if not all(bits[f] for f in fbits):
                continue
            acc = np.ones(128, dtype=np.int64)
            for pi in pbits:
                acc &= (p >> (6 - pi)) & 1
            vec *= 1.0 - 2.0 * acc  # pp/ pf pairs; pure-ff: acc==1 -> -1
        k = tuple((key >> (nb - 1 - i)) & 1 for i in range(nb))
        region_vecs[k] = None if np.all(vec == 1.0) else vec.astype(np.float32)
    return split_bits, region_vecs


# ------------------------- device program -------------------------

_NC_CACHE = {}


def _build_nc(reps=1, metas_shape=None):
    """metas_shape: list of (stype, split_bits, region_has_vec) describing the
    evac decoration structure (device program depends on structure only)."""
    import concourse.bacc as bacc
    import concourse.mybir as mybir
    import concourse.tile as tile

    F32 = mybir.dt.float32
    F32R = mybir.dt.float32r
    BF16 = mybir.dt.bfloat16
    SDT = BF16 if STATE_BF16 else F32R
    AX = mybir.AluOpType
    ACTF = mybir.ActivationFunctionType

    nc = bacc.Bacc(None)
    pr = nc.declare_dram_parameter("pr", [128, 8192], SDT, isOutput=False)
    pi = nc.declare_dram_parameter("pi", [128, 8192], SDT, isOutput=False)
    wps = [nc.declare_dram_parameter(f"w{s}", [128, 128], SDT, isOutput=False)
           for s in range(len(STAGES))]
    # per-stage region sign vectors, packed as [128, n_regions] f32
    svps = {}
    for s, (stype, split_bits, region_has_vec) in enumerate(metas_shape):
        nvec = sum(1 for h in region_has_vec.values() if h)
        if nvec:
            svps[s] = nc.declare_dram_parameter(f"sv{s}", [128, nvec], F32,
                                                isOutput=False)
    uvec = nc.declare_dram_parameter("uvec", [128, 1], F32, isOutput=False)
    maskA = nc.declare_dram_parameter("maskA", [128, 1], F32, isOutput=False)
    ones64 = nc.declare_dram_parameter("ones64", [64, 128], F32, isOutput=False)
    out = nc.declare_dram_parameter("out", [128, 16384], F32, isOutput=True)

    with tile.TileContext(nc) as tc:
        with (
            tc.tile_pool(name="st", bufs=1) as stp,
            tc.tile_pool(name="wp", bufs=1) as wp,
            tc.tile_pool(name="small", bufs=1) as smp,
            tc.tile_pool(name="pstm", bufs=8, space="PSUM") as pstm,
        ):
            Af = stp.tile([128, 16384], SDT, tag="A")
            Bf = stp.tile([128, 16384], SDT, tag="B")
            A = Af[:].rearrange("p (c f) -> p c f", c=2)
            Bv = Bf[:].rearrange("p (c f) -> p c f", c=2)
            # f32 scratch for measurement/output staging
            Sc = stp.tile([128, 16384], F32, tag="S")

            # one-time loads
            wts = []
            for s in range(len(STAGES)):
                wt = wp.tile([128, 128], SDT, tag=f"w{s}")
                nc.gpsimd.dma_start(wt[:], wps[s][:])
                wts.append(wt)
            svts = {}
            for s, dram in svps.items():
                t = wp.tile([128, dram.shape[1]], F32, tag=f"sv{s}")
                nc.gpsimd.dma_start(t[:], dram[:])
                svts[s] = t
            for ch in range(8):
                sl = slice(ch * 1024, (ch + 1) * 1024)
                nc.sync.dma_start(A[:, 0, sl], pr[:, sl])
                nc.sync.dma_start(A[:, 1, sl], pi[:, sl])

            # --- evac helper ---------------------------------------------
            # unit_bits: list of (bitpos, dim_name) describing free-axis bit
            # positions that are constant per evac unit; within-unit splits
            # handled via view rearranges by the caller providing split specs.
            def emit_evac(eng_is_dve, pv, dv, scale_vec):
                """one op: copy / per-partition-vec multiply."""
                if scale_vec is None:
                    if eng_is_dve:
                        nc.vector.tensor_copy(dv, pv)
                    else:
                        nc.scalar.copy(dv, pv)
                else:
                    if eng_is_dve:
                        nc.vector.tensor_scalar(dv, pv, scale_vec, None,
                                                op0=AX.mult)
                    else:
                        nc.scalar.mul(dv, pv, scale_vec)

            def split_views(pv, dv, dim_bits, want_bits):
                """Split views along free bits that live inside this unit.
                pv/dv: AP views with identical logical dims [p, ...dims].
                dim_bits: mapping bitpos -> (dim_index, stride_within_dim)
                want_bits: subset of bitpos to split on.
                Yields (region_bits_dict, pv_sub, dv_sub)."""
                if not want_bits:
                    yield {}, pv, dv
                    return
                bit = want_bits[0]
                rest = want_bits[1:]
                dim, stride = dim_bits[bit]
                # split dim of size n into (n//(2*stride), 2, stride)
                def sp(view):
                    shp = view.shape
                    n = shp[dim]
                    pre = list(range(len(shp)))
                    # build rearrange string
                    names = [f"d{i}" for i in range(len(shp))]
                    src = " ".join(
                        names[i] if i != dim else "(u v w)"
                        for i in range(len(shp)))
                    dst = " ".join(
                        names[i] if i != dim else "u v w"
                        for i in range(len(shp)))
                    return view.rearrange(f"{src} -> {dst}",
                                          v=2, w=stride)
                pvs, dvs = sp(pv), sp(dv)
                for v in (0, 1):
                    idx = tuple(
                        slice(None) if i != dim else None
                        for i in range(len(pv.shape)))
                    pv_sub = pvs[tuple(
                        [slice(None)] * dim + [slice(None), v])]
                    # the above is wrong for generic dims; handled by caller
                    raise RuntimeError("unused")

            # Simplified: the stage emitters below handle splits directly.

            def region_sign(svt, region_cols, key):
                col = region_cols.get(key)
                if col is None:
                    return None
                return svt[:, col:col + 1]

            # --- stage emitters -------------------------------------------
            def tm_stage(src, dst, w, deco, svt, region_cols, unit_par=2):
                """TM6: out free index bits: 0..5 = blk (pr_*2+b), 6..12 = x.
                psum tile [128,512] = [b(2), c(2), x(128)]."""
                split_bits, _ = deco
                # bits constant per tile: bit < 5 (pr_ bits: pr_ is 5 bits) ;
                # bit 5 = b ; bits 6..12 = x bits
                tile_bits = [f for f in split_bits if f < 5]
                b_bits = [f for f in split_bits if f == 5]
                x_bits = [f for f in split_bits if f >= 6]
                for pr_ in range(32):
                    p = pstm.tile([128, 512], F32, tag="tm")
                    for b in range(2):
                        blk = pr_ * 2 + b
                        for c in range(2):
                            nc.tensor.matmul(
                                p[:, (b * 2 + c) * 128:(b * 2 + c + 1) * 128],
                                src[:, c, blk * 128:(blk + 1) * 128],
                                w[:], start=True, stop=True)
                    pv = p[:].rearrange("p (b c x) -> p b c x", b=2, c=2)
                    dv = dst[:, :, pr_ * 256:(pr_ + 1) * 256].rearrange(
                        "p c (b x) -> p b c x", b=2)
                    eng_is_dve = (pr_ % 2 == 0)
                    base_key = {f: (pr_ >> (4 - f)) & 1 for f in tile_bits}
                    if not b_bits and not x_bits:
                        key = tuple(base_key[f] for f in split_bits)
                        emit_evac(eng_is_dve, pv, dv,
                                  region_sign(svt, region_cols, key))
                    else:
                        for bb in ((0, 1) if b_bits else (None,)):
                            pvb = pv if bb is None else pv[:, bb:bb + 1]
                            dvb = dv if bb is None else dv[:, bb:bb + 1]
                            if x_bits:
                                assert len(x_bits) == 1
                                xb = x_bits[0]
                                stride = 1 << (12 - xb)
                                u = 128 // (2 * stride)
                                pvx = pvb.rearrange(
                                    "p b c (u v w) -> p b c u v w",
                                    v=2, w=stride)
                                dvx = dvb.rearrange(
                                    "p b c (u v w) -> p b c u v w",
                                    v=2, w=stride)
                                for vv in (0, 1):
                                    kd = dict(base_key)
                                    if bb is not None:
                                        kd[5] = bb
                                    kd[xb] = vv
                                    key = tuple(kd[f] for f in split_bits)
                                    emit_evac(
                                        eng_is_dve,
                                        pvx[:, :, :, :, vv:vv + 1],
                                        dvx[:, :, :, :, vv:vv + 1],
                                        region_sign(svt, region_cols, key))
                            else:
                                kd = dict(base_key)
                                kd[5] = bb
                                key = tuple(kd[f] for f in split_bits)
                                emit_evac(eng_is_dve, pvb, dvb,
                                          region_sign(svt, region_cols, key))

            def tm0_stage(src, dst, w, deco, svt, region_cols):
                """TM0: out free index = p_old(7 bits: dim w) * 64 + l(6 bits:
                blk = pr_*2+b). bits 0..6 = w bits, 7..12 = blk bits."""
                split_bits, _ = deco
                srcr = src[:, 0, :].rearrange("p (w l) -> p l w", l=64)
                srci = src[:, 1, :].rearrange("p (w l) -> p l w", l=64)
                dstv = dst.rearrange("p c (w l) -> p l c w", l=64)
                blk_bits = [f for f in split_bits if f >= 7]
                w_bits = [f for f in split_bits if f < 7]
                for pr_ in range(32):
                    p = pstm.tile([128, 512], F32, tag="tm")
                    for b in range(2):
                        blk = pr_ * 2 + b
                        nc.tensor.matmul(p[:, (b * 2) * 128:(b * 2 + 1) * 128],
                                         srcr[:, blk, :], w[:],
                                         start=True, stop=True)
                        nc.tensor.matmul(
                            p[:, (b * 2 + 1) * 128:(b * 2 + 2) * 128],
                            srci[:, blk, :], w[:], start=True, stop=True)
                    pv = p[:].rearrange("p (b c x) -> p b c x", b=2, c=2)
                    dv = dstv[:, pr_ * 2:pr_ * 2 + 2, :, :]
                    eng_is_dve = (pr_ % 2 == 0)
                    # blk = pr_*2+b: bit 7+k of free = bit of blk: blk bits:
                    # blk is 6 bits (0..63): free bit 7+j = blk bit j (j=0 MSB)
                    base_key = {}
                    for f in blk_bits:
                        j = f - 7  # blk bit index, 0 = MSB of 6
                        if j < 5:
                            base_key[f] = (pr_ >> (4 - j)) & 1
                    b_in_blk = [f for f in blk_bits if f - 7 == 5]
                    for bb in ((0, 1) if b_in_blk else (None,)):
                        pvb = pv if bb is None else pv[:, bb:bb + 1]
                        dvb = dv if bb is None else dv[:, bb:bb + 1]
                        if w_bits:
                            assert len(w_bits) == 1
                            wb = w_bits[0]
                            stride = 1 << (6 - wb)  # within w dim (128 vals)
                            pvx = pvb.rearrange(
                                "p b c (u v z) -> p b c u v z",
                                v=2, z=stride)
                            dvx = dvb.rearrange(
                                "p l c (u v z) -> p l c u v z",
                                v=2, z=stride)
                            for vv in (0, 1):
                                kd = dict(base_key)
                                if bb is not None:
                                    kd[b_in_blk[0]] = bb
                                kd[wb] = vv
                                key = tuple(kd[f] for f in split_bits)
                                emit_evac(eng_is_dve,
                                          pvx[:, :, :, :, vv:vv + 1],
                                          dvx[:, :, :, :, vv:vv + 1],
                                          region_sign(svt, region_cols, key))
                        else:
                            kd = dict(base_key)
                            if bb is not None:
                                kd[b_in_blk[0]] = bb
                            key = tuple(kd[f] for f in split_bits)
                            emit_evac(eng_is_dve, pvb, dvb,
                                      region_sign(svt, region_cols, key))

            def pm_stage(src, dst, w, deco, svt, region_cols):
                """PM: free index unchanged: bits 0..3 = chunk (16 chunks of
                512), bits 4..12 within chunk."""
                split_bits, _ = deco
                ch_bits = [f for f in split_bits if f < 4]
                in_bits = [f for f in split_bits if f >= 4]
                for ch in range(16):
                    sl = slice(ch * 512, (ch + 1) * 512)
                    pre = pstm.tile([128, 512], F32, tag="tm")
                    pim = pstm.tile([128, 512], F32, tag="tm")
                    nc.tensor.matmul(pre[:], w[:], src[:, 0, sl],
                                     start=True, stop=True)
                    nc.tensor.matmul(pim[:], w[:], src[:, 1, sl],
                                     start=True, stop=True)
                    base_key = {f: (ch >> (3 - f)) & 1 for f in ch_bits}
                    for c, pp in ((0, pre), (1, pim)):
                        eng_is_dve = ((ch * 2 + c) % 2 == 0)
                        pv = pp[:]
                        dv = dst[:, c, sl]
                        if in_bits:
                            assert len(in_bits) == 1
                            ib = in_bits[0]
                            stride = 1 << (12 - ib)
                            pvx = pv.rearrange("p (u v z) -> p u v z",
                                               v=2, z=stride)
                            dvx = dv.rearrange("p (u v z) -> p u v z",
                                               v=2, z=stride)
                            for vv in (0, 1):
                                kd = dict(base_key)
                                kd[ib] = vv
                                key = tuple(kd[f] for f in split_bits)
                                emit_evac(eng_is_dve,
                                          pvx[:, :, vv:vv + 1],
                                          dvx[:, :, vv:vv + 1],
                                          region_sign(svt, region_cols, key))
                        else:
                            key = tuple(base_key[f] for f in split_bits)
                            emit_evac(eng_is_dve, pv, dv,
                                      region_sign(svt, region_cols, key))

            # --- region column maps (host-fixed ordering) ------------------
            region_cols_all = []
            for s, (stype, split_bits, region_has_vec) in enumerate(
                    metas_shape):
                cols = {}
                ci = 0
                for key in sorted(region_has_vec.keys()):
                    if region_has_vec[key]:
                        cols[key] = ci
                        ci += 1
                region_cols_all.append(cols)

            cur, nxt = A, Bv
            for _rep in range(reps):
                for s, (stype, layer) in enumerate(STAGES):
                    split_bits = metas_shape[s][1]
                    deco = (split_bits, None)
                    svt = svts.get(s)
                    rc = region_cols_all[s]
                    if stype == "TM6":
                        tm_stage(cur, nxt, wts[s], deco, svt, rc)
                    elif stype == "TM0":
                        tm0_stage(cur, nxt, wts[s], deco, svt, rc)
                    else:
                        pm_stage(cur, nxt, wts[s], deco, svt, rc)
                    cur, nxt = nxt, cur
            assert cur is A

            # ---- measurement on qubit 0 (partition MSB; partitions 0..63)
            acc = smp.tile([64, 4], F32, tag="acc")
            scr_r = Sc[0:64, 0:8192]
            scr_i = Sc[0:64, 8192:16384]
            nc.scalar.activation(scr_r, A[0:64, 0, :], ACTF.Square,
                                 accum_out=acc[:, 0:1])
            nc.vector.scalar_tensor_tensor(scr_i, A[0:64, 1, :], 1.0,
                                           A[0:64, 1, :], op0=AX.bypass,
                                           op1=AX.mult, accum_out=acc[:, 1:2])
            nc.vector.tensor_add(acc[:, 2:3], acc[:, 0:1], acc[:, 1:2])
            o64 = smp.tile([64, 128], F32, tag="ones")
            nc.gpsimd.dma_start(o64[:], ones64[:])
            pp0 = pstm.tile([128, 1], F32, tag="tm")
            nc.tensor.matmul(pp0[:], o64[:], acc[:, 2:3], start=True, stop=True)

            sm = smp.tile([128, 12], F32, tag="sm")
            p0v, tv, a1, a2, pv_, rv, invv, omt, s0, s1, diff, S = (
                sm[:, k:k + 1] for k in range(12))
            uvt = smp.tile([128, 1], F32, tag="uv")
            mAt = smp.tile([128, 1], F32, tag="mA")
            nc.gpsimd.dma_start(uvt[:], uvec[:])
            nc.gpsimd.dma_start(mAt[:], maskA[:])
            nc.vector.tensor_copy(p0v, pp0[:])
            nc.vector.tensor_tensor(tv, uvt[:], p0v, op=AX.is_ge)
            nc.vector.tensor_scalar(a1, p0v, -2.0, 1.0, op0=AX.mult, op1=AX.add)
            nc.vector.tensor_tensor(a2, tv, a1, op=AX.mult)
            nc.vector.tensor_tensor(pv_, p0v, a2, op=AX.add)
            nc.vector.reciprocal(rv, pv_)
            nc.scalar.sqrt(invv, rv)
            nc.vector.tensor_scalar(omt, tv, -1.0, 1.0, op0=AX.mult, op1=AX.add)
            nc.vector.tensor_tensor(s0, invv, omt, op=AX.mult)
            nc.vector.tensor_tensor(s1, invv, tv, op=AX.mult)
            nc.vector.tensor_tensor(diff, s0, s1, op=AX.subtract)
            nc.vector.tensor_tensor(a2, mAt[:], diff, op=AX.mult)
            nc.vector.tensor_tensor(S, s1, a2, op=AX.add)

            # ---- interleave re/im with scale, then DMA out
            Spair = Sc[:].rearrange("p (f c) -> p f c", c=2)
            for ch in range(8):
                fsl = slice(ch * 1024, (ch + 1) * 1024)
                nc.vector.tensor_scalar(Spair[:, fsl, 0], A[:, 0, fsl], S,
                                        None, op0=AX.mult)
                nc.scalar.mul(Spair[:, fsl, 1], A[:, 1, fsl], S)
                osl = slice(ch * 2048, (ch + 1) * 2048)
                nc.sync.dma_start(out[:, osl], Sc[:, osl])
    nc.compile()
    return nc


def _shape_key(metas):
    shape = []
    for m in metas:
        split_bits, region_vecs = stage_decorations(m)
        has = {k: (v is not None) for k, v in region_vecs.items()}
        shape.append((m["stype"], tuple(split_bits),
                      tuple(sorted(has.items()))))
    return tuple(shape)


def _get_nc(reps, metas):
    shape = []
    for m in metas:
        split_bits, region_vecs = stage_decorations(m)
        has = {k: (v is not None) for k, v in region_vecs.items()}
        shape.append((m["stype"], list(split_bits), has))
    key = (reps, _shape_key(metas))
    if key not in _NC_CACHE:
        _NC_CACHE[key] = _build_nc(reps, shape)
    return _NC_CACHE[key]


# ------------------------- entry point -------------------------

def _popcount_diag():
    idx = np.arange(DIM, dtype=np.int64)
    pc = np.zeros(DIM, dtype=np.int64)
    for q in range(N):
        pc += (idx >> q) & 1
    return pc & 3


def _cz_sign_canonical():
    idx = np.arange(DIM, dtype=np.int64)
    bits = (idx[None, :] >> (N - 1 - np.arange(N)[:, None])) & 1
    par = np.sum(bits[:-1] * bits[1:], axis=0) % 2
    return (1 - 2 * par).astype(np.float32)


def kernel(psi_re, psi_im, thetas, u, _trace=False):
    from concourse.bass_utils import run_bass_kernel_spmd
    import ml_dtypes

    psi_re = np.asarray(psi_re, dtype=np.float32)
    psi_im = np.asarray(psi_im, dtype=np.float32)
    thetas = np.asarray(thetas, dtype=np.float32)
    u = np.asarray(u, dtype=np.float32)

    metas = build_plan_v2(thetas.astype(np.float64))

    # host pre: chi = S^dag psi ; S^dag diag = (-i)^popcount
    k4 = _popcount_diag()  # popcount mod 4
    # (-i)^k: k=0: (re,im); 1: (im,-re); 2: (-re,-im); 3: (-im,re)
    cr = np.where(k4 == 0, 1.0, np.where(k4 == 2, -1.0, 0.0)).astype(np.float32)
    ci = np.where(k4 == 1, -1.0, np.where(k4 == 3, 1.0, 0.0)).astype(np.float32)
    # chi = (cr + i*ci) * (re + i*im) -> chi_re = cr*re - ci*im ; chi_im =
    # cr*im + ci*re
    chi_re = cr[None, :] * psi_re - ci[None, :] * psi_im
    chi_im = cr[None, :] * psi_im + ci[None, :] * psi_re

    sdt = ml_dtypes.bfloat16 if STATE_BF16 else np.float32

    nc = _get_nc(1, metas)
    maskA = (np.arange(128) < 64).astype(np.float32).reshape(128, 1)
    ones64 = np.ones((64, 128), dtype=np.float32)

    # weights: U^T per stage
    wts = [np.ascontiguousarray(m["U"].T.astype(np.float64)).astype(sdt)
           for m in metas]
    svs = {}
    for s, m in enumerate(metas):
        split_bits, region_vecs = stage_decorations(m)
        cols = [v for k, v in sorted(region_vecs.items()) if v is not None]
        if cols:
            svs[s] = np.stack(cols, axis=1).astype(np.float32)

    in_maps = []
    for b in range(BATCH):
        mdict = {
            "pr": chi_re[b].reshape(128, 8192).astype(sdt),
            "pi": chi_im[b].reshape(128, 8192).astype(sdt),
            "uvec": np.full((128, 1), u[b], dtype=np.float32),
            "maskA": maskA,
            "ones64": ones64,
        }
        for s in range(len(STAGES)):
            mdict[f"w{s}"] = wts[s]
        for s, sv in svs.items():
            mdict[f"sv{s}"] = sv
        in_maps.append(mdict)

    res = run_bass_kernel_spmd(nc, in_maps, list(range(BATCH)), trace=_trace)

    # host post: psi_out = S * CZ * phi ; T = i^popcount * czsign
    cz = _cz_sign_canonical()
    tr = np.where(k4 == 0, 1.0, np.where(k4 == 2, -1.0, 0.0)).astype(np.float32) * cz
    ti = np.where(k4 == 1, 1.0, np.where(k4 == 3, -1.0, 0.0)).astype(np.float32) * cz
    outs = []
    for b in range(BATCH):
        o = res.results[b]["out"].reshape(DIM, 2)
        fr = tr * o[:, 0] - ti * o[:, 1]
        fi = tr * o[:, 1] + ti * o[:, 0]
        outs.append(np.stack([fr, fi], axis=-1))
    return np.stack(outs).astype(np.float32)
